# revision 28
# baseline (speedup 1.0000x reference)
"""Trainium2 Bass kernel for nn_Classifier_3788161155197.

Structure (per core, SPMD over 8 cores, no cross-core communication):
  rows [c*512 - W, c*512 + 512 + W) window (halo W each side)
  A) context LSTM cell (zero state -> only W_ih terms; f-gate unused),
     attention block skipped (softmax row-sums are exactly 1, so
     sent_encoding == outp2), inner = tanh(outp2 @ ip_w.T + b),
     discourse input gates P = inner @ dW_ih.T + db  (both directions)
  B) discourse bidirectional LSTM, parallelized as 128+ chunks x len 4
     per direction with W warmup steps (state decay ~0.5/step makes
     chunking error small vs bf16 matmul noise). Sequence edges
     handled by forcing i/f gates to -40 on padded rows (exact state
     reset), so core 0/7 warmups reproduce the true zero init.
     Per (step, dir): one 4-bank PSUM tile in gate-major order
     (i,f,o,g)x(4 kk) -> fused pf-add (1 DVE op), sigmoid over i,f,o
     (1 ACT), tanh over g (1 ACT); h written straight into the strided
     hs buffer which doubles as next step's matmul rhs.
  C) sliding maxpool(+-2) as 2 fused DVE maxes + pred as 25 matmuls
     accumulating into a [2, 512] PSUM tile (FD=512, 2-col weights).
All matmuls bf16 operands with fp32 PSUM accumulation.
"""

import numpy as np
import ml_dtypes

import concourse.bass as bass
import concourse.bacc as bacc
import concourse.tile as tile
import concourse.mybir as mybir
from concourse.bass_utils import run_bass_kernel_spmd

AF = mybir.ActivationFunctionType
ALU = mybir.AluOpType
BF16 = mybir.dt.bfloat16
F32 = mybir.dt.float32

N, E, H = 4096, 768, 512
NC = 8
S = N // NC            # 512 rows per core
W = 4                  # warmup steps / halo
L = 4                  # chunk length
TS = W + L + 2         # recurrence steps per direction (incl +2 tail)
WN = S + 2 * W         # window columns
NJ = WN // L           # chunk count (j dim)
NT = 2                 # n-tiles in phase A
NTW = WN // NT
KE = E // 128          # 6 K-chunks over embedding
KH2 = (2 * H) // 128   # 8 K-chunks over 2H
PAD = W                # masked head/tail columns for gate reset
HJ = (W + 3) // L      # j-columns touched by head/tail pads
JB = tuple((W - l0 + 3) // 4 for l0 in range(4))   # first j of each l block
R0 = tuple(4 * JB[l0] + l0 - W for l0 in range(4))  # first row of each block
BIGPOS = 60000.0
GRESET = -40.0
NEGBIG = -3.0e38

_cache = {}


def _split_waits(nc):
    """Walrus (this build) accepts at most ONE sem wait per instruction and
    does not split Tile's multi-wait sync_infos itself. Hoist excess waits
    onto injected same-engine NoOps placed immediately before."""
    cnt = 0
    for f in nc.m.functions:
        for bb in f.blocks:
            insts = bb.instructions
            i = 0
            while i < len(insts):
                inst = insts[i]
                si = inst.sync_info
                if si is not None and si.on_wait and len(si.on_wait) > 1:
                    waits = list(si.on_wait)
                    for w in waits[:-1]:
                        n = mybir.InstNoOp(name=f"wsplit-{cnt}", ins=[], outs=[])
                        cnt += 1
                        n.engine = inst.engine
                        n.sync_info = mybir.SyncInfo(on_wait=[w], on_update=[])
                        insts.insert(i, n)
                        i += 1
                    inst.sync_info = mybir.SyncInfo(
                        on_wait=[waits[-1]], on_update=list(si.on_update or []))
                i += 1
    return cnt


def _bf16(x):
    return np.asarray(x, np.float32).astype(ml_dtypes.bfloat16)


def _wtiles(w_np):
    """[M,K] weight -> [128, M/128, K/128, 128] bf16 with
    arr[p,m,k,q] = w[m*128+q, k*128+p] (lhsT tiles for out = x @ w.T)."""
    M, K = w_np.shape
    nm, nk = M // 128, K // 128
    return _bf16(w_np.reshape(nm, 128, nk, 128).transpose(3, 0, 2, 1).copy())


def _btiles(b_np):
    """[M] bias -> [128, M/128] fp32."""
    M = b_np.shape[0]
    return np.ascontiguousarray(b_np.reshape(M // 128, 128).T.astype(np.float32))


def _ifog(w):
    """Reorder PyTorch gate rows (i,f,g,o) -> (i,f,o,g)."""
    return np.concatenate([w[0:H], w[H:2 * H], w[3 * H:4 * H], w[2 * H:3 * H]])


def _build():
    nc = bacc.Bacc("TRN2", target_bir_lowering=False, debug=False)

    def din(name, shape, dt):
        return nc.dram_tensor(name, shape, dt, kind="ExternalInput").ap()

    sent = din("sent", [128, KE, WN], BF16)
    cwf = din("cwf", [128, 12, KE, 128], BF16)   # ctx W_ih.T tiles, gates i,g,o
    cwb = din("cwb", [128, 12, KE, 128], BF16)
    ipw = din("ipw", [128, KE, KH2, 128], BF16)  # ip_w tiles [M=768 rows, K=1024]
    dwf = din("dwf", [128, 16, KE, 128], BF16)   # disc W_ih.T tiles, (i,f,o,g)
    dwb = din("dwb", [128, 16, KE, 128], BF16)
    whf = din("whf", [128, 16, 4, 128], BF16)    # W_hh tiles, (i,f,o,g)
    whb = din("whb", [128, 16, 4, 128], BF16)
    # biases packed: cbf[12] cbb[12] ipb[6] dbf[16] dbb[16]
    bias = din("bias", [128, 62], F32)
    apad = din("apad", [128, 8, 4, 2 * HJ], BF16)  # i/f gate clamp, l-major edges
    hpad = din("hpad", [128, 2, 4, 2], BF16)     # maxpool edge mask cols W-1, W+S
    dfeat = din("dfeat", [16, 4, 128], BF16)     # disc_feat.T + ones row, l-major
    pwm = din("pwm", [128, 24, 2], BF16)         # pred_w.T main K-chunks
    pwd = din("pwd", [16, 2], BF16)              # pred_w.T disc rows + bias row
    pred_o = nc.dram_tensor("pred", [2, 4, 128], F32, kind="ExternalOutput").ap()

    def dma(dst, src):
        return nc.gpsimd.dma_start(dst, src)

    with tile.TileContext(nc) as tc:
        with (
            tc.tile_pool(name="const", bufs=1) as cpool,
            tc.tile_pool(name="acts", bufs=1) as apool,
            tc.tile_pool(name="wstream", bufs=6) as wpool,
            tc.tile_pool(name="tmp", bufs=2) as tpool,
            tc.tile_pool(name="tmp1", bufs=1) as t1pool,
        ):
            # ---- resident loads (batched: few big DMAs, critical first) ----
            warmsrc = cpool.tile([128, 640], BF16)
            nc.gpsimd.memset(warmsrc[:], 0.0)
            sent_sb = cpool.tile([128, KE, WN], BF16)
            dma(sent_sb[:], sent[:])
            bias_sb = cpool.tile([128, 62], F32)
            dma(bias_sb[:], bias[:])
            cbf_sb = bias_sb[:, 0:12]
            cbb_sb = bias_sb[:, 12:24]
            ipb_sb = bias_sb[:, 24:30]
            dbf_sb = bias_sb[:, 30:46]
            dbb_sb = bias_sb[:, 46:62]
            whf_sb = cpool.tile([128, 16, 4, 128], BF16)
            whb_sb = cpool.tile([128, 16, 4, 128], BF16)
            apad_sb = cpool.tile([128, 8, 4, 2 * HJ], BF16)
            hpad_sb = cpool.tile([128, 2, 4, 2], BF16)
            dfeat_sb = cpool.tile([16, 4, 128], BF16)
            pwm_sb = cpool.tile([128, 24, 2], BF16)
            pwd_sb = cpool.tile([16, 2], BF16)

            hout = apool.tile([128, KH2, WN], BF16)   # outp2.T chunks (f0-3,b0-3)
            inner = apool.tile([128, KE, WN], BF16)   # inner.T chunks
            # discourse input gates per dir, gate-major (i,f,o,g)x(kk)
            pf = {d: apool.tile([128, 16, WN], F32, tag=f"pf{d}", name=f"pf{d}")
                  for d in "fb"}
            hs = apool.tile([128, 2, 4, WN], BF16, name="hs")

            # ---- phase A: context gates -> h -> outp2 ----
            with tc.tile_pool(name="psA", bufs=4, space="PSUM") as psA:
                # HAM warm-up: dependency-free matmuls on scratch data keep
                # the PE busy (and its clock at 2.4GHz) during initial DMAs.
                wps = psA.tile([128, 512], F32, tag="warm", bufs=1)
                for _ in range(16):
                    nc.tensor.matmul(wps[:], warmsrc[:, 0:128],
                                     warmsrc[:, 128:640], start=True, stop=True)
                for d, cw_d, cb_sb in (("f", cwf, cbf_sb), ("b", cwb, cbb_sb)):
                    for kk in range(4):
                        gt = {}
                        for gi, g in enumerate(("i", "g", "o")):
                            m = gi * 4 + kk
                            wt = wpool.tile([128, KE, 128], BF16, tag="w")
                            dma(wt[:], cw_d[:, m])
                            gs = tpool.tile([128, WN], BF16, tag=f"cg{g}")
                            for n in range(NT):
                                ps = psA.tile([128, NTW], F32, tag="ps")
                                for k in range(KE):
                                    nc.tensor.matmul(
                                        ps[:],
                                        wt[:, k],
                                        sent_sb[:, k, n * NTW:(n + 1) * NTW],
                                        start=(k == 0), stop=(k == KE - 1))
                                fn = AF.Tanh if g == "g" else AF.Sigmoid
                                nc.scalar.activation(
                                    gs[:, n * NTW:(n + 1) * NTW], ps[:], fn,
                                    bias=cb_sb[:, m:m + 1])
                            gt[g] = gs
                        cprod = tpool.tile([128, WN], BF16, tag="cprod")
                        nc.vector.tensor_mul(cprod[:], gt["i"][:], gt["g"][:])
                        tc_ = tpool.tile([128, WN], BF16, tag="tanc")
                        nc.scalar.activation(tc_[:], cprod[:], AF.Tanh)
                        hchunk = (0 if d == "f" else 4) + kk
                        nc.vector.tensor_mul(
                            hout[:, hchunk], gt["o"][:], tc_[:])

                # ---- inner = tanh(outp2 @ ip_w.T + b) ----
                for m in range(KE):
                    wt = wpool.tile([128, KH2, 128], BF16, tag="wip")
                    dma(wt[:], ipw[:, m])
                    for n in range(NT):
                        ps = psA.tile([128, NTW], F32, tag="ps")
                        for k in range(KH2):
                            nc.tensor.matmul(
                                ps[:], wt[:, k],
                                hout[:, k, n * NTW:(n + 1) * NTW],
                                start=(k == 0), stop=(k == KH2 - 1))
                        nc.scalar.activation(
                            inner[:, m, n * NTW:(n + 1) * NTW], ps[:], AF.Tanh,
                            bias=ipb_sb[:, m:m + 1])

                dma(whf_sb[:], whf[:])
                dma(whb_sb[:], whb[:])
                dma(apad_sb[:], apad[:])
                dma(hpad_sb[:], hpad[:])
                dma(dfeat_sb[:], dfeat[:])
                dma(pwm_sb[:], pwm[:])
                dma(pwd_sb[:], pwd[:])
                # ---- discourse input gates ----
                for d, dw_d, db_sb in (("f", dwf, dbf_sb), ("b", dwb, dbb_sb)):
                    for m in range(16):
                        wt = wpool.tile([128, KE, 128], BF16, tag="w")
                        dma(wt[:], dw_d[:, m])
                        for n in range(NT):
                            ps = psA.tile([128, NTW], F32, tag="ps")
                            for k in range(KE):
                                nc.tensor.matmul(
                                    ps[:], wt[:, k],
                                    inner[:, k, n * NTW:(n + 1) * NTW],
                                    start=(k == 0), stop=(k == KE - 1))
                            nc.scalar.activation(
                                pf[d][:, m, n * NTW:(n + 1) * NTW],
                                ps[:], AF.Identity, bias=db_sb[:, m:m + 1])
                    # exact state reset on padded rows: i/f gates -> -40
                    # (pad cols live in the first/last 3 j of each l block)
                    pfe = pf[d][:].rearrange("p m (l j) -> p m l j", j=NJ)
                    nc.vector.tensor_tensor(
                        pfe[:, 0:8, :, 0:HJ], pfe[:, 0:8, :, 0:HJ],
                        apad_sb[:, :, :, 0:HJ], ALU.min)
                    nc.vector.tensor_tensor(
                        pfe[:, 0:8, :, NJ - HJ:NJ], pfe[:, 0:8, :, NJ - HJ:NJ],
                        apad_sb[:, :, :, HJ:2 * HJ], ALU.min)

            # ---- phase B: chunked recurrences (f and b interleaved) ----
            z16 = apool.tile([128, 512], BF16)
            nc.gpsimd.memset(z16[:], 0.0)
            cst = {d: apool.tile([128, 4, 128], BF16, tag=f"c{d}", name=f"cst{d}")
                   for d in "fb"}
            nc.gpsimd.memset(cst["f"][:], 0.0)
            nc.gpsimd.memset(cst["b"][:], 0.0)

            pfv = {d: pf[d][:].rearrange("p m (l j) -> p m l j", j=NJ) for d in "fb"}
            # hs stored l-major: col c lives at (l=c%L, j=c//L), address l*NJ+j,
            # so each step's h-write and next step's matmul rhs are dense.
            hsv = hs[:].rearrange("p d k (l j) -> p d k l j", j=NJ)
            z16v = z16[:].rearrange("p (k b) -> p k b", k=4)

            def off_of(d, t):
                return t if d == "f" else (2 * W + 3 - t)

            with tc.tile_pool(name="psB", bufs=1, space="PSUM") as psB:
                prev = {}
                pstile = {d: psB.tile([128, 16, 128], F32, tag=f"ps{d}",
                                      name=f"ps{d}")
                          for d in "fb"}
                # psum gate blocks: i=0:4, f=4:8, o=8:12, g=12:16; emit the
                # f block first so sigmoid(f) and the c-chain start early.
                GB = {"f": (4, 8), "i": (0, 4), "g": (12, 16), "o": (8, 12)}

                def mmblock(ps, wh_sb, di, d, t, gate):
                    for m in range(*GB[gate]):
                        for k in range(4):
                            if t == 0:
                                rhs = z16v[:, k]
                            else:
                                pph, pj0 = prev[d]
                                rhs = hsv[:, di, k, pph, pj0:pj0 + 128]
                            # t=0 initializes has_written; t>0 accumulates
                            # onto the pf values pre-copied into PSUM.
                            nc.tensor.matmul(
                                ps[:, m], wh_sb[:, m, k], rhs,
                                start=(k == 0 and t == 0), stop=(k == 3),
                                skip_group_check=True)

                for t in range(TS):
                    for di, (d, wh_sb) in enumerate((("f", whf_sb), ("b", whb_sb))):
                        off = off_of(d, t)
                        ph, j0 = off % L, off // L
                        ps = pstile[d]
                        ifo = t1pool.tile([128, 12, 128], BF16, tag=f"ifo{d}",
                                          bufs=2)
                        gg = t1pool.tile([128, 4, 128], BF16, tag=f"gg{d}",
                                         bufs=2)
                        c = cst[d]
                        it = t1pool.tile([128, 4, 128], BF16, tag=f"it{d}")

                        def pfadd0(gate):
                            if t == 0:
                                gl, gh = GB[gate]
                                nc.vector.tensor_tensor(
                                    ps[:, gl:gh], ps[:, gl:gh],
                                    pfv[d][:, gl:gh, ph, j0:j0 + 128], ALU.add)

                        def stage(gate):
                            # stage next step's input gates for this block into
                            # PSUM right after its activation has read it
                            if t < TS - 1:
                                gl, gh = GB[gate]
                                noff = off_of(d, t + 1)
                                nph, nj0 = noff % L, noff // L
                                nc.vector.tensor_copy(
                                    ps[:, gl:gh],
                                    pfv[d][:, gl:gh, nph, nj0:nj0 + 128])

                        mmblock(ps, wh_sb, di, d, t, "f")
                        pfadd0("f")
                        nc.scalar.activation(ifo[:, 4:8], ps[:, 4:8], AF.Sigmoid)
                        nc.vector.tensor_mul(c[:], ifo[:, 4:8], c[:])
                        stage("f")
                        mmblock(ps, wh_sb, di, d, t, "i")
                        pfadd0("i")
                        nc.scalar.activation(ifo[:, 0:4], ps[:, 0:4], AF.Sigmoid)
                        stage("i")
                        mmblock(ps, wh_sb, di, d, t, "g")
                        pfadd0("g")
                        nc.scalar.activation(gg[:], ps[:, 12:16], AF.Tanh)
                        nc.vector.tensor_mul(it[:], ifo[:, 0:4], gg[:])
                        nc.vector.tensor_add(c[:], c[:], it[:])
                        stage("g")
                        mmblock(ps, wh_sb, di, d, t, "o")
                        pfadd0("o")
                        nc.scalar.activation(ifo[:, 8:12], ps[:, 8:12], AF.Sigmoid)
                        stage("o")
                        tch = t1pool.tile([128, 4, 128], BF16, tag=f"tc{d}",
                                          bufs=2)
                        nc.scalar.activation(tch[:], c[:], AF.Tanh)
                        # h = o * tanh(c), written straight into l-major hs
                        nc.vector.tensor_mul(
                            hsv[:, di, :, ph, j0:j0 + 128], ifo[:, 8:12], tch[:])
                        prev[d] = (ph, j0)

            # maxpool edge masking (only nonzero for cores 0/7)
            cb, ca = W - 1, W + S
            nc.vector.tensor_add(
                hsv[:, :, :, cb % 4, cb // 4:cb // 4 + 1],
                hsv[:, :, :, cb % 4, cb // 4:cb // 4 + 1],
                hpad_sb[:, :, :, 0:1])
            nc.vector.tensor_add(
                hsv[:, :, :, ca % 4, ca // 4:ca // 4 + 1],
                hsv[:, :, :, ca % 4, ca // 4:ca // 4 + 1],
                hpad_sb[:, :, :, 1:2])

            # ---- phase C: maxpool + pred ----
            # output col c = 4*(JB[l0]+i) + l0 for i in 0..127 (row r = c-W)
            # (l, j-shift) pairs addressing cols c-1, c-2 (before) / c+1, c+2
            BEF = {0: ((3, -1), (2, -1)), 1: ((0, 0), (3, -1)),
                   2: ((1, 0), (0, 0)), 3: ((2, 0), (1, 0))}
            AFT = {0: ((1, 0), (2, 0)), 1: ((2, 0), (3, 0)),
                   2: ((3, 0), (0, 1)), 3: ((0, 1), (1, 1))}
            mb = apool.tile([128, 2, 4, 4, 128], BF16)
            ma = apool.tile([128, 2, 4, 4, 128], BF16)
            for l0 in range(4):
                j0 = JB[l0]
                for dst, src in ((mb, BEF), (ma, AFT)):
                    (la, sa), (lb, sb) = src[l0]
                    nc.vector.tensor_max(
                        dst[:, :, :, l0],
                        hsv[:, :, :, la, j0 + sa:j0 + sa + 128],
                        hsv[:, :, :, lb, j0 + sb:j0 + sb + 128])

            pred_sb = apool.tile([2, 4, 128], F32)
            with tc.tile_pool(name="psC", bufs=1, space="PSUM") as psC:
                ps = psC.tile([2, 4, 128], F32, tag="pp")
                # "i" (raw hs) first: those matmuls don't wait on the maxpool
                for l0 in range(4):
                    first = True
                    for grp in ("i", "b", "a"):
                        ci = {"b": 0, "a": 8, "i": 16}[grp]
                        for di in range(2):
                            for kk in range(4):
                                if grp == "i":
                                    rhs = hsv[:, di, kk, l0,
                                              JB[l0]:JB[l0] + 128]
                                elif grp == "b":
                                    rhs = mb[:, di, kk, l0]
                                else:
                                    rhs = ma[:, di, kk, l0]
                                nc.tensor.matmul(
                                    ps[:, l0], pwm_sb[:, ci], rhs,
                                    start=first, stop=False)
                                first = False
                                ci += 1
                    nc.tensor.matmul(
                        ps[:, l0], pwd_sb[:], dfeat_sb[:, l0],
                        start=False, stop=True)
                nc.vector.tensor_copy(pred_sb[:], ps[:])
            dma(pred_o[:], pred_sb[:])
    nc.finalize()
    return nc


def _prep(inputs):
    """Host-side prep -> per-core in_maps (shared arrays reused across cores)."""
    sent_T = np.asarray(inputs["sentence"], np.float32)  # [N, E]

    shared = {}
    biases = np.zeros((128, 62), np.float32)
    for d in "fb":
        w = np.asarray(inputs[f"cW_ih_{d}"], np.float32)
        b = np.asarray(inputs[f"cb_{d}"], np.float32)
        # context: keep gates i,g,o (f unused with zero state)
        sel = np.concatenate([w[0:H], w[2 * H:3 * H], w[3 * H:4 * H]])
        bsel = np.concatenate([b[0:H], b[2 * H:3 * H], b[3 * H:4 * H]])
        shared["cwf" if d == "f" else "cwb"] = _wtiles(sel)
        cb = _btiles(bsel)
        if d == "f":
            biases[:, 0:12] = cb
        else:
            biases[:, 12:24] = cb
        # discourse: reorder gates to (i,f,o,g)
        shared["dwf" if d == "f" else "dwb"] = _wtiles(
            _ifog(np.asarray(inputs[f"dW_ih_{d}"], np.float32)))
        db = _btiles(_ifog(np.asarray(inputs[f"db_{d}"], np.float32)))
        if d == "f":
            biases[:, 30:46] = db
        else:
            biases[:, 46:62] = db
        shared["whf" if d == "f" else "whb"] = _wtiles(
            _ifog(np.asarray(inputs[f"dW_hh_{d}"], np.float32)))
    shared["ipw"] = _wtiles(np.asarray(inputs["ip_w"], np.float32))
    biases[:, 24:30] = _btiles(np.asarray(inputs["ip_b"], np.float32))
    shared["bias"] = biases

    pw = np.asarray(inputs["pred_w"], np.float32)  # [2, 6H+9]
    pb = np.asarray(inputs["pred_b"], np.float32)
    pwm = pw[:, :6 * H].T.reshape(24, 128, 2).transpose(1, 0, 2)
    shared["pwm"] = _bf16(np.ascontiguousarray(pwm))
    pwd = np.zeros((16, 2), np.float32)
    pwd[:9] = pw[:, 6 * H:].T
    pwd[9] = pb
    shared["pwd"] = _bf16(pwd)

    disc = np.asarray(inputs["disc_feat"], np.float32)

    # l-major column order: position (l, j) holds window col j*L + l
    perm = (np.arange(NJ)[None, :] * L + np.arange(L)[:, None]).ravel()

    in_maps = []
    for c in range(NC):
        lo = c * S
        hl = lo - W
        m = dict(shared)
        win = np.zeros((WN, E), np.float32)
        a, b_ = max(0, hl), min(N, hl + WN)
        win[a - hl:b_ - hl] = sent_T[a:b_]
        win = win[perm]
        m["sent"] = _bf16(win.reshape(WN, KE, 128).transpose(2, 1, 0).copy())

        pad = np.zeros(WN, bool)
        rows = hl + np.arange(WN)
        pad[(rows < 0) | (rows >= N)] = True
        ap = np.where(pad, GRESET, BIGPOS).astype(np.float32)
        ap_l = ap[perm].reshape(4, NJ)
        apc = np.concatenate([ap_l[:, 0:HJ], ap_l[:, NJ - HJ:NJ]], axis=1)
        m["apad"] = _bf16(np.broadcast_to(apc, (128, 8, 4, 2 * HJ)).copy())
        hp = np.zeros(2, np.float32)
        if c == 0:
            hp[0] = NEGBIG       # col W-1 = row -1
        if c == NC - 1:
            hp[1] = NEGBIG       # col W+S = row N (=4096)
        m["hpad"] = _bf16(np.broadcast_to(hp, (128, 2, 4, 2)).copy())

        df = np.zeros((16, S), np.float32)
        df[:9] = disc[lo:lo + S].T
        df[9] = 1.0
        # l-major column permutation: block l0 covers rows R0[l0]::4
        dfp = np.zeros((16, 4, 128), np.float32)
        for l0 in range(4):
            dfp[:, l0] = df[:, R0[l0]::4]
        m["dfeat"] = _bf16(dfp)
        in_maps.append(m)
    return in_maps


def kernel(**inputs):
    if "nc" not in _cache:
        _cache["nc"] = _build()
    in_maps = _prep(inputs)
    res = run_bass_kernel_spmd(_cache["nc"], in_maps, list(range(NC)))
    out = np.empty((N, 2), np.float32)
    for c in range(NC):
        p = res.results[c]["pred"]  # [2, 4, 128], l-major columns
        for l0 in range(4):
            out[c * S + R0[l0]:(c + 1) * S:4] = p[:, l0].T
    return out


# revision 29
# speedup vs baseline: 1.0090x; 1.0090x over previous
"""Trainium2 Bass kernel for nn_Classifier_3788161155197.

Structure (per core, SPMD over 8 cores, no cross-core communication):
  rows [c*512 - W, c*512 + 512 + W) window (halo W each side)
  A) context LSTM cell (zero state -> only W_ih terms; f-gate unused),
     attention block skipped (softmax row-sums are exactly 1, so
     sent_encoding == outp2), inner = tanh(outp2 @ ip_w.T + b),
     discourse input gates P = inner @ dW_ih.T + db  (both directions)
  B) discourse bidirectional LSTM, parallelized as 128+ chunks x len 4
     per direction with W warmup steps (state decay ~0.5/step makes
     chunking error small vs bf16 matmul noise). Sequence edges
     handled by forcing i/f gates to -40 on padded rows (exact state
     reset), so core 0/7 warmups reproduce the true zero init.
     Per (step, dir): one 4-bank PSUM tile in gate-major order
     (i,f,o,g)x(4 kk) -> fused pf-add (1 DVE op), sigmoid over i,f,o
     (1 ACT), tanh over g (1 ACT); h written straight into the strided
     hs buffer which doubles as next step's matmul rhs.
  C) sliding maxpool(+-2) as 2 fused DVE maxes + pred as 25 matmuls
     accumulating into a [2, 512] PSUM tile (FD=512, 2-col weights).
All matmuls bf16 operands with fp32 PSUM accumulation.
"""

import numpy as np
import ml_dtypes

import concourse.bass as bass
import concourse.bacc as bacc
import concourse.tile as tile
import concourse.mybir as mybir
from concourse.bass_utils import run_bass_kernel_spmd

AF = mybir.ActivationFunctionType
ALU = mybir.AluOpType
BF16 = mybir.dt.bfloat16
F32 = mybir.dt.float32

N, E, H = 4096, 768, 512
NC = 8
S = N // NC            # 512 rows per core
W = 4                  # warmup steps / halo
L = 4                  # chunk length
TS = W + L + 2         # recurrence steps per direction (incl +2 tail)
WN = S + 2 * W         # window columns
NJ = WN // L           # chunk count (j dim)
NT = 2                 # n-tiles in phase A
NTW = WN // NT
KE = E // 128          # 6 K-chunks over embedding
KH2 = (2 * H) // 128   # 8 K-chunks over 2H
PAD = W                # masked head/tail columns for gate reset
HJ = (W + 3) // L      # j-columns touched by head/tail pads
JB = tuple((W - l0 + 3) // 4 for l0 in range(4))   # first j of each l block
R0 = tuple(4 * JB[l0] + l0 - W for l0 in range(4))  # first row of each block
BIGPOS = 60000.0
GRESET = -40.0
NEGBIG = -3.0e38

_cache = {}


def _split_waits(nc):
    """Walrus (this build) accepts at most ONE sem wait per instruction and
    does not split Tile's multi-wait sync_infos itself. Hoist excess waits
    onto injected same-engine NoOps placed immediately before."""
    cnt = 0
    for f in nc.m.functions:
        for bb in f.blocks:
            insts = bb.instructions
            i = 0
            while i < len(insts):
                inst = insts[i]
                si = inst.sync_info
                if si is not None and si.on_wait and len(si.on_wait) > 1:
                    waits = list(si.on_wait)
                    for w in waits[:-1]:
                        n = mybir.InstNoOp(name=f"wsplit-{cnt}", ins=[], outs=[])
                        cnt += 1
                        n.engine = inst.engine
                        n.sync_info = mybir.SyncInfo(on_wait=[w], on_update=[])
                        insts.insert(i, n)
                        i += 1
                    inst.sync_info = mybir.SyncInfo(
                        on_wait=[waits[-1]], on_update=list(si.on_update or []))
                i += 1
    return cnt


def _bf16(x):
    return np.asarray(x, np.float32).astype(ml_dtypes.bfloat16)


def _wtiles(w_np):
    """[M,K] weight -> [128, M/128, K/128, 128] bf16 with
    arr[p,m,k,q] = w[m*128+q, k*128+p] (lhsT tiles for out = x @ w.T)."""
    M, K = w_np.shape
    nm, nk = M // 128, K // 128
    return _bf16(w_np.reshape(nm, 128, nk, 128).transpose(3, 0, 2, 1).copy())


def _btiles(b_np):
    """[M] bias -> [128, M/128] fp32."""
    M = b_np.shape[0]
    return np.ascontiguousarray(b_np.reshape(M // 128, 128).T.astype(np.float32))


def _ifog(w):
    """Reorder PyTorch gate rows (i,f,g,o) -> (i,f,o,g)."""
    return np.concatenate([w[0:H], w[H:2 * H], w[3 * H:4 * H], w[2 * H:3 * H]])


def _build():
    nc = bacc.Bacc("TRN2", target_bir_lowering=False, debug=False)

    def din(name, shape, dt):
        return nc.dram_tensor(name, shape, dt, kind="ExternalInput").ap()

    sent = din("sent", [128, KE, WN], BF16)
    cwf = din("cwf", [128, 12, KE, 128], BF16)   # ctx W_ih.T tiles, gates i,g,o
    cwb = din("cwb", [128, 12, KE, 128], BF16)
    ipw = din("ipw", [128, KE, KH2, 128], BF16)  # ip_w tiles [M=768 rows, K=1024]
    dwf = din("dwf", [128, 16, KE, 128], BF16)   # disc W_ih.T tiles, (i,f,o,g)
    dwb = din("dwb", [128, 16, KE, 128], BF16)
    whf = din("whf", [128, 16, 4, 128], BF16)    # W_hh tiles, (i,f,o,g)
    whb = din("whb", [128, 16, 4, 128], BF16)
    # biases packed: cbf[12] cbb[12] ipb[6] dbf[16] dbb[16]
    bias = din("bias", [128, 62], F32)
    apad = din("apad", [128, 8, 4, 2 * HJ], BF16)  # i/f gate clamp, l-major edges
    hpad = din("hpad", [128, 2, 4, 2], BF16)     # maxpool edge mask cols W-1, W+S
    dfeat = din("dfeat", [16, 4, 128], BF16)     # disc_feat.T + ones row, l-major
    pwm = din("pwm", [128, 24, 2], BF16)         # pred_w.T main K-chunks
    pwd = din("pwd", [16, 2], BF16)              # pred_w.T disc rows + bias row
    pred_o = nc.dram_tensor("pred", [2, 4, 128], F32, kind="ExternalOutput").ap()

    def dma(dst, src):
        return nc.gpsimd.dma_start(dst, src)

    with tile.TileContext(nc) as tc:
        with (
            tc.tile_pool(name="const", bufs=1) as cpool,
            tc.tile_pool(name="acts", bufs=1) as apool,
            tc.tile_pool(name="wstream", bufs=6) as wpool,
            tc.tile_pool(name="tmp", bufs=2) as tpool,
            tc.tile_pool(name="tmp1", bufs=1) as t1pool,
        ):
            # ---- resident loads (batched: few big DMAs, critical first) ----
            warmsrc = cpool.tile([128, 640], BF16)
            nc.gpsimd.memset(warmsrc[:], 0.0)
            sent_sb = cpool.tile([128, KE, WN], BF16)
            dma(sent_sb[:], sent[:])
            bias_sb = cpool.tile([128, 62], F32)
            dma(bias_sb[:], bias[:])
            cbf_sb = bias_sb[:, 0:12]
            cbb_sb = bias_sb[:, 12:24]
            ipb_sb = bias_sb[:, 24:30]
            dbf_sb = bias_sb[:, 30:46]
            dbb_sb = bias_sb[:, 46:62]
            whf_sb = cpool.tile([128, 16, 4, 128], BF16)
            whb_sb = cpool.tile([128, 16, 4, 128], BF16)
            apad_sb = cpool.tile([128, 8, 4, 2 * HJ], BF16)
            hpad_sb = cpool.tile([128, 2, 4, 2], BF16)
            dfeat_sb = cpool.tile([16, 4, 128], BF16)
            pwm_sb = cpool.tile([128, 24, 2], BF16)
            pwd_sb = cpool.tile([16, 2], BF16)

            hout = apool.tile([128, KH2, WN], BF16)   # outp2.T chunks (f0-3,b0-3)
            inner = apool.tile([128, KE, WN], BF16)   # inner.T chunks
            # discourse input gates per dir, gate-major (i,f,o,g)x(kk)
            pf = {d: apool.tile([128, 16, WN], BF16, tag=f"pf{d}", name=f"pf{d}")
                  for d in "fb"}
            hs = apool.tile([128, 2, 4, WN], BF16, name="hs")

            # ---- phase A: context gates -> h -> outp2 ----
            with tc.tile_pool(name="psA", bufs=4, space="PSUM") as psA:
                # HAM warm-up: dependency-free matmuls on scratch data keep
                # the PE busy (and its clock at 2.4GHz) during initial DMAs.
                wps = psA.tile([128, 512], F32, tag="warm", bufs=1)
                for _ in range(16):
                    nc.tensor.matmul(wps[:], warmsrc[:, 0:128],
                                     warmsrc[:, 128:640], start=True, stop=True)
                for d, cw_d, cb_sb in (("f", cwf, cbf_sb), ("b", cwb, cbb_sb)):
                    for kk in range(4):
                        gt = {}
                        for gi, g in enumerate(("i", "g", "o")):
                            m = gi * 4 + kk
                            wt = wpool.tile([128, KE, 128], BF16, tag="w")
                            dma(wt[:], cw_d[:, m])
                            gs = tpool.tile([128, WN], BF16, tag=f"cg{g}")
                            for n in range(NT):
                                ps = psA.tile([128, NTW], F32, tag="ps")
                                for k in range(KE):
                                    nc.tensor.matmul(
                                        ps[:],
                                        wt[:, k],
                                        sent_sb[:, k, n * NTW:(n + 1) * NTW],
                                        start=(k == 0), stop=(k == KE - 1))
                                fn = AF.Tanh if g == "g" else AF.Sigmoid
                                nc.scalar.activation(
                                    gs[:, n * NTW:(n + 1) * NTW], ps[:], fn,
                                    bias=cb_sb[:, m:m + 1])
                            gt[g] = gs
                        cprod = tpool.tile([128, WN], BF16, tag="cprod")
                        nc.vector.tensor_mul(cprod[:], gt["i"][:], gt["g"][:])
                        tc_ = tpool.tile([128, WN], BF16, tag="tanc")
                        nc.scalar.activation(tc_[:], cprod[:], AF.Tanh)
                        hchunk = (0 if d == "f" else 4) + kk
                        nc.vector.tensor_mul(
                            hout[:, hchunk], gt["o"][:], tc_[:])

                # ---- inner = tanh(outp2 @ ip_w.T + b) ----
                for m in range(KE):
                    wt = wpool.tile([128, KH2, 128], BF16, tag="wip")
                    dma(wt[:], ipw[:, m])
                    for n in range(NT):
                        ps = psA.tile([128, NTW], F32, tag="ps")
                        for k in range(KH2):
                            nc.tensor.matmul(
                                ps[:], wt[:, k],
                                hout[:, k, n * NTW:(n + 1) * NTW],
                                start=(k == 0), stop=(k == KH2 - 1))
                        nc.scalar.activation(
                            inner[:, m, n * NTW:(n + 1) * NTW], ps[:], AF.Tanh,
                            bias=ipb_sb[:, m:m + 1])

                dma(whf_sb[:], whf[:])
                dma(whb_sb[:], whb[:])
                dma(apad_sb[:], apad[:])
                dma(hpad_sb[:], hpad[:])
                dma(dfeat_sb[:], dfeat[:])
                dma(pwm_sb[:], pwm[:])
                dma(pwd_sb[:], pwd[:])
                # ---- discourse input gates ----
                for d, dw_d, db_sb in (("f", dwf, dbf_sb), ("b", dwb, dbb_sb)):
                    for m in range(16):
                        wt = wpool.tile([128, KE, 128], BF16, tag="w")
                        dma(wt[:], dw_d[:, m])
                        for n in range(NT):
                            ps = psA.tile([128, NTW], F32, tag="ps")
                            for k in range(KE):
                                nc.tensor.matmul(
                                    ps[:], wt[:, k],
                                    inner[:, k, n * NTW:(n + 1) * NTW],
                                    start=(k == 0), stop=(k == KE - 1))
                            nc.scalar.activation(
                                pf[d][:, m, n * NTW:(n + 1) * NTW],
                                ps[:], AF.Identity, bias=db_sb[:, m:m + 1])
                    # exact state reset on padded rows: i/f gates -> -40
                    # (pad cols live in the first/last 3 j of each l block)
                    pfe = pf[d][:].rearrange("p m (l j) -> p m l j", j=NJ)
                    nc.vector.tensor_tensor(
                        pfe[:, 0:8, :, 0:HJ], pfe[:, 0:8, :, 0:HJ],
                        apad_sb[:, :, :, 0:HJ], ALU.min)
                    nc.vector.tensor_tensor(
                        pfe[:, 0:8, :, NJ - HJ:NJ], pfe[:, 0:8, :, NJ - HJ:NJ],
                        apad_sb[:, :, :, HJ:2 * HJ], ALU.min)

            # ---- phase B: chunked recurrences (f and b interleaved) ----
            z16 = apool.tile([128, 512], BF16)
            nc.gpsimd.memset(z16[:], 0.0)
            cst = {d: apool.tile([128, 4, 128], BF16, tag=f"c{d}", name=f"cst{d}")
                   for d in "fb"}
            nc.gpsimd.memset(cst["f"][:], 0.0)
            nc.gpsimd.memset(cst["b"][:], 0.0)

            pfv = {d: pf[d][:].rearrange("p m (l j) -> p m l j", j=NJ) for d in "fb"}
            # hs stored l-major: col c lives at (l=c%L, j=c//L), address l*NJ+j,
            # so each step's h-write and next step's matmul rhs are dense.
            hsv = hs[:].rearrange("p d k (l j) -> p d k l j", j=NJ)
            z16v = z16[:].rearrange("p (k b) -> p k b", k=4)

            def off_of(d, t):
                return t if d == "f" else (2 * W + 3 - t)

            with tc.tile_pool(name="psB", bufs=1, space="PSUM") as psB:
                prev = {}
                pstile = {d: psB.tile([128, 16, 128], F32, tag=f"ps{d}",
                                      name=f"ps{d}")
                          for d in "fb"}
                # psum gate blocks: i=0:4, f=4:8, o=8:12, g=12:16; emit the
                # f block first so sigmoid(f) and the c-chain start early.
                GB = {"f": (4, 8), "i": (0, 4), "g": (12, 16), "o": (8, 12)}

                def mmblock(ps, wh_sb, di, d, t, gate):
                    for m in range(*GB[gate]):
                        for k in range(4):
                            if t == 0:
                                rhs = z16v[:, k]
                            else:
                                pph, pj0 = prev[d]
                                rhs = hsv[:, di, k, pph, pj0:pj0 + 128]
                            # t=0 initializes has_written; t>0 accumulates
                            # onto the pf values pre-copied into PSUM.
                            nc.tensor.matmul(
                                ps[:, m], wh_sb[:, m, k], rhs,
                                start=(k == 0 and t == 0), stop=(k == 3),
                                skip_group_check=True)

                for t in range(TS):
                    for di, (d, wh_sb) in enumerate((("f", whf_sb), ("b", whb_sb))):
                        off = off_of(d, t)
                        ph, j0 = off % L, off // L
                        ps = pstile[d]
                        ifo = t1pool.tile([128, 12, 128], BF16, tag=f"ifo{d}",
                                          bufs=2)
                        gg = t1pool.tile([128, 4, 128], BF16, tag=f"gg{d}",
                                         bufs=2)
                        c = cst[d]
                        it = t1pool.tile([128, 4, 128], BF16, tag=f"it{d}")

                        def pfadd0(gate):
                            if t == 0:
                                gl, gh = GB[gate]
                                nc.vector.tensor_tensor(
                                    ps[:, gl:gh], ps[:, gl:gh],
                                    pfv[d][:, gl:gh, ph, j0:j0 + 128], ALU.add)

                        def stage(gate, eng):
                            # stage next step's input gates for this block into
                            # PSUM right after its activation has read it
                            if t < TS - 1:
                                gl, gh = GB[gate]
                                noff = off_of(d, t + 1)
                                nph, nj0 = noff % L, noff // L
                                src = pfv[d][:, gl:gh, nph, nj0:nj0 + 128]
                                if eng == "v":
                                    nc.vector.tensor_copy(ps[:, gl:gh], src)
                                else:
                                    nc.scalar.activation(
                                        ps[:, gl:gh], src, AF.Copy)

                        mmblock(ps, wh_sb, di, d, t, "f")
                        pfadd0("f")
                        nc.scalar.activation(ifo[:, 4:8], ps[:, 4:8], AF.Sigmoid)
                        nc.vector.tensor_mul(c[:], ifo[:, 4:8], c[:])
                        stage("f", "v")
                        mmblock(ps, wh_sb, di, d, t, "i")
                        pfadd0("i")
                        nc.scalar.activation(ifo[:, 0:4], ps[:, 0:4], AF.Sigmoid)
                        stage("i", "v")
                        mmblock(ps, wh_sb, di, d, t, "g")
                        pfadd0("g")
                        nc.scalar.activation(gg[:], ps[:, 12:16], AF.Tanh)
                        nc.vector.tensor_mul(it[:], ifo[:, 0:4], gg[:])
                        nc.vector.tensor_add(c[:], c[:], it[:])
                        stage("g", "s")
                        mmblock(ps, wh_sb, di, d, t, "o")
                        pfadd0("o")
                        nc.scalar.activation(ifo[:, 8:12], ps[:, 8:12], AF.Sigmoid)
                        stage("o", "s")
                        tch = t1pool.tile([128, 4, 128], BF16, tag=f"tc{d}",
                                          bufs=2)
                        nc.scalar.activation(tch[:], c[:], AF.Tanh)
                        # keep the PE activity monitor warm through the chain
                        # wait: tiny loads timed by mid-chain deps (no PSUM use)
                        nc.tensor.ldweights(it[:, 0, 0:8])
                        nc.tensor.ldweights(tch[:, 0, 0:8])
                        # h = o * tanh(c), written straight into l-major hs
                        nc.vector.tensor_mul(
                            hsv[:, di, :, ph, j0:j0 + 128], ifo[:, 8:12], tch[:])
                        prev[d] = (ph, j0)

            # maxpool edge masking (only nonzero for cores 0/7)
            cb, ca = W - 1, W + S
            nc.vector.tensor_add(
                hsv[:, :, :, cb % 4, cb // 4:cb // 4 + 1],
                hsv[:, :, :, cb % 4, cb // 4:cb // 4 + 1],
                hpad_sb[:, :, :, 0:1])
            nc.vector.tensor_add(
                hsv[:, :, :, ca % 4, ca // 4:ca // 4 + 1],
                hsv[:, :, :, ca % 4, ca // 4:ca // 4 + 1],
                hpad_sb[:, :, :, 1:2])

            # ---- phase C: maxpool + pred ----
            # output col c = 4*(JB[l0]+i) + l0 for i in 0..127 (row r = c-W)
            # (l, j-shift) pairs addressing cols c-1, c-2 (before) / c+1, c+2
            BEF = {0: ((3, -1), (2, -1)), 1: ((0, 0), (3, -1)),
                   2: ((1, 0), (0, 0)), 3: ((2, 0), (1, 0))}
            AFT = {0: ((1, 0), (2, 0)), 1: ((2, 0), (3, 0)),
                   2: ((3, 0), (0, 1)), 3: ((0, 1), (1, 1))}
            mb = apool.tile([128, 2, 4, 4, 128], BF16)
            ma = apool.tile([128, 2, 4, 4, 128], BF16)
            for l0 in range(4):
                j0 = JB[l0]
                for dst, src in ((mb, BEF), (ma, AFT)):
                    (la, sa), (lb, sb) = src[l0]
                    nc.vector.tensor_max(
                        dst[:, :, :, l0],
                        hsv[:, :, :, la, j0 + sa:j0 + sa + 128],
                        hsv[:, :, :, lb, j0 + sb:j0 + sb + 128])

            pred_sb = apool.tile([2, 4, 128], F32)
            with tc.tile_pool(name="psC", bufs=1, space="PSUM") as psC:
                ps = psC.tile([2, 4, 128], F32, tag="pp")
                # "i" (raw hs) first: those matmuls don't wait on the maxpool
                for l0 in range(4):
                    first = True
                    for grp in ("i", "b", "a"):
                        ci = {"b": 0, "a": 8, "i": 16}[grp]
                        for di in range(2):
                            for kk in range(4):
                                if grp == "i":
                                    rhs = hsv[:, di, kk, l0,
                                              JB[l0]:JB[l0] + 128]
                                elif grp == "b":
                                    rhs = mb[:, di, kk, l0]
                                else:
                                    rhs = ma[:, di, kk, l0]
                                nc.tensor.matmul(
                                    ps[:, l0], pwm_sb[:, ci], rhs,
                                    start=first, stop=False)
                                first = False
                                ci += 1
                    nc.tensor.matmul(
                        ps[:, l0], pwd_sb[:], dfeat_sb[:, l0],
                        start=False, stop=True)
                nc.vector.tensor_copy(pred_sb[:], ps[:])
            dma(pred_o[:], pred_sb[:])
    nc.finalize()
    return nc


def _prep(inputs):
    """Host-side prep -> per-core in_maps (shared arrays reused across cores)."""
    sent_T = np.asarray(inputs["sentence"], np.float32)  # [N, E]

    shared = {}
    biases = np.zeros((128, 62), np.float32)
    for d in "fb":
        w = np.asarray(inputs[f"cW_ih_{d}"], np.float32)
        b = np.asarray(inputs[f"cb_{d}"], np.float32)
        # context: keep gates i,g,o (f unused with zero state)
        sel = np.concatenate([w[0:H], w[2 * H:3 * H], w[3 * H:4 * H]])
        bsel = np.concatenate([b[0:H], b[2 * H:3 * H], b[3 * H:4 * H]])
        shared["cwf" if d == "f" else "cwb"] = _wtiles(sel)
        cb = _btiles(bsel)
        if d == "f":
            biases[:, 0:12] = cb
        else:
            biases[:, 12:24] = cb
        # discourse: reorder gates to (i,f,o,g)
        shared["dwf" if d == "f" else "dwb"] = _wtiles(
            _ifog(np.asarray(inputs[f"dW_ih_{d}"], np.float32)))
        db = _btiles(_ifog(np.asarray(inputs[f"db_{d}"], np.float32)))
        if d == "f":
            biases[:, 30:46] = db
        else:
            biases[:, 46:62] = db
        shared["whf" if d == "f" else "whb"] = _wtiles(
            _ifog(np.asarray(inputs[f"dW_hh_{d}"], np.float32)))
    shared["ipw"] = _wtiles(np.asarray(inputs["ip_w"], np.float32))
    biases[:, 24:30] = _btiles(np.asarray(inputs["ip_b"], np.float32))
    shared["bias"] = biases

    pw = np.asarray(inputs["pred_w"], np.float32)  # [2, 6H+9]
    pb = np.asarray(inputs["pred_b"], np.float32)
    pwm = pw[:, :6 * H].T.reshape(24, 128, 2).transpose(1, 0, 2)
    shared["pwm"] = _bf16(np.ascontiguousarray(pwm))
    pwd = np.zeros((16, 2), np.float32)
    pwd[:9] = pw[:, 6 * H:].T
    pwd[9] = pb
    shared["pwd"] = _bf16(pwd)

    disc = np.asarray(inputs["disc_feat"], np.float32)

    # l-major column order: position (l, j) holds window col j*L + l
    perm = (np.arange(NJ)[None, :] * L + np.arange(L)[:, None]).ravel()

    in_maps = []
    for c in range(NC):
        lo = c * S
        hl = lo - W
        m = dict(shared)
        win = np.zeros((WN, E), np.float32)
        a, b_ = max(0, hl), min(N, hl + WN)
        win[a - hl:b_ - hl] = sent_T[a:b_]
        win = win[perm]
        m["sent"] = _bf16(win.reshape(WN, KE, 128).transpose(2, 1, 0).copy())

        pad = np.zeros(WN, bool)
        rows = hl + np.arange(WN)
        pad[(rows < 0) | (rows >= N)] = True
        ap = np.where(pad, GRESET, BIGPOS).astype(np.float32)
        ap_l = ap[perm].reshape(4, NJ)
        apc = np.concatenate([ap_l[:, 0:HJ], ap_l[:, NJ - HJ:NJ]], axis=1)
        m["apad"] = _bf16(np.broadcast_to(apc, (128, 8, 4, 2 * HJ)).copy())
        hp = np.zeros(2, np.float32)
        if c == 0:
            hp[0] = NEGBIG       # col W-1 = row -1
        if c == NC - 1:
            hp[1] = NEGBIG       # col W+S = row N (=4096)
        m["hpad"] = _bf16(np.broadcast_to(hp, (128, 2, 4, 2)).copy())

        df = np.zeros((16, S), np.float32)
        df[:9] = disc[lo:lo + S].T
        df[9] = 1.0
        # l-major column permutation: block l0 covers rows R0[l0]::4
        dfp = np.zeros((16, 4, 128), np.float32)
        for l0 in range(4):
            dfp[:, l0] = df[:, R0[l0]::4]
        m["dfeat"] = _bf16(dfp)
        in_maps.append(m)
    return in_maps


def kernel(**inputs):
    if "nc" not in _cache:
        _cache["nc"] = _build()
    in_maps = _prep(inputs)
    res = run_bass_kernel_spmd(_cache["nc"], in_maps, list(range(NC)))
    out = np.empty((N, 2), np.float32)
    for c in range(NC):
        p = res.results[c]["pred"]  # [2, 4, 128], l-major columns
        for l0 in range(4):
            out[c * S + R0[l0]:(c + 1) * S:4] = p[:, l0].T
    return out
